# revision 56
# baseline (speedup 1.0000x reference)
"""Trainium2 Bass kernel for nn_DigitConvolutionalModel (dense CNN -> MLP).

Pure data parallel over 8 NeuronCores (2048 samples each). The 3x3 conv is
linear, so the host folds it into the first FC layer (W1e = C @ w1.T), making
the whole network a 4-layer MLP computed in transposed orientation (features
on partitions, batch on the free dim) in fp16 (psum fp32, ~5e-4 rel err):

    outT = w4e.T @ relu3(w3t.T @ relu(w2t.T @ relu(W1e.T @ xT + b1) + b2) + b3)

where relu3 appends a constant ones row so w4e's extra row adds b4 inside the
L4 matmul; the final output is a pure fp32 copy PSUM->SBUF then DMA.

Raw bass with manual semaphores. Both HW DGE queues (sync + scalar) carry the
input stream in need order so arrivals pace the PE; warmup matmuls keep the
PE busy from the preamble's end (the DVFS governor only reaches 2.4GHz after
a long continuous-busy stretch, and the PE must never idle >0.5us). Batch
tiles are [512,512,512,256,256]: the two half-width trailing tiles halve
every op in the serial end chain (relu/copy on ACT/DVE + L2-L4 matmuls),
shrinking the critical tail after the last x byte lands. All PSUM accesses
stay bank-aligned at offset 0 (this hardware faults on intra-bank offsets).

PE op order (A=L1, B=L2, C=L3, D=L4):
  A0 A1 B0 A2 C0 B1 A3 D0 C1 B2 A4 D1 C2 B3 D2 C3 B4 D3 C4 D4
L1 relu halves run on ACT (m=0) and DVE (m=1) concurrently; h2 on DVE;
h3 and the out copies on ACT. s2 counts PE tail ops (B/C/D) in PE order.
"""

from contextlib import ExitStack

import ml_dtypes
import numpy as np

import concourse.bass as bass
import concourse.mybir as mybir

N_CORES = 8
B = 16384
BC = B // N_CORES
KC = 112
NKC = 7

TS = [512, 512, 512, 256, 256]          # batch tile sizes
OFF = [sum(TS[:t]) for t in range(len(TS))]
NT = len(TS)
NBMAX = max(TS)

F32 = mybir.dt.float32
BF16 = mybir.dt.bfloat16
FP16 = mybir.dt.float16
RELU = mybir.ActivationFunctionType.Relu
ADD = mybir.AluOpType.add
MAX = mybir.AluOpType.max

N_WARM_MM = 13

# x DMA chunk splits per tile (in units of the tile's contraction chunks).
X_SPLITS = [
    [(c, c + 1) for c in range(NKC)],
    [(0, 2), (2, 4), (4, 6), (6, 7)],
    [(0, 2), (2, 4), (4, 6), (6, 7)],
    [(0, 4), (4, 7)],
    [(0, 4), (4, 7)],
]

# A4 is emitted late so the PE's final L1 tile overlaps the earlier tiles'
# cross-engine relu/copy chains instead of idling behind them.
PE_ORDER = [
    ("A", 0), ("A", 1), ("B", 0), ("A", 2), ("C", 0), ("B", 1), ("A", 3),
    ("D", 0), ("C", 1), ("B", 2), ("D", 1), ("C", 2), ("B", 3), ("D", 2),
    ("A", 4), ("C", 3), ("D", 3), ("B", 4), ("C", 4), ("D", 4),
]
TAILS = [(k, t) for (k, t) in PE_ORDER if k != "A"]
POS_PE = {op: i + 1 for i, op in enumerate(TAILS)}  # s2 thresholds

# ACT ops (r0 = L1 relu first half; h3; out = ps4->osb copy), in readiness
# order (by the PE-emission index of each op's producer).
ACT_ORDER = [
    ("r0", 0), ("r0", 1), ("r0", 2), ("h3", 0), ("r0", 3), ("out", 0),
    ("h3", 1), ("out", 1), ("h3", 2), ("out", 2), ("r0", 4), ("h3", 3),
    ("out", 3), ("h3", 4), ("out", 4),
]
POS_A = {op: i + 1 for i, op in enumerate(ACT_ORDER)}  # sa thresholds

# DVE ops (r1 = L1 relu second half; h2), in readiness order.
DVE_ORDER = [
    ("r1", 0), ("r1", 1), ("h2", 0), ("r1", 2), ("h2", 1), ("r1", 3),
    ("h2", 2), ("h2", 3), ("r1", 4), ("h2", 4),
]
POS_V = {op: i + 1 for i, op in enumerate(DVE_ORDER)}  # sv thresholds


def build_program(l1_dt=FP16, l234_dt=FP16):
    nc = bass.Bass()

    n_wp = 256 + 64 + 10
    ncol = NKC * BC

    xt_d = nc.declare_dram_parameter("xt", [KC, ncol], l1_dt, isOutput=False)
    w1_d = nc.declare_dram_parameter("w1e", [KC, 2, NKC * 128], l1_dt, isOutput=False)
    wp_d = nc.declare_dram_parameter("wpack", [128, n_wp], l234_dt, isOutput=False)
    bp_d = nc.declare_dram_parameter("bpack", [128, 4], F32, isOutput=False)
    out_d = nc.declare_dram_parameter("outT", [10, BC], F32, isOutput=True)

    def col(t, c):
        return NKC * OFF[t] + c * TS[t]

    ctx = ExitStack()
    with ctx:
        xsb = ctx.enter_context(nc.sbuf_tensor([KC, ncol], l1_dt))
        w1sb = ctx.enter_context(nc.sbuf_tensor([KC, 2, NKC, 128], l1_dt))
        wpsb = ctx.enter_context(nc.sbuf_tensor([128, n_wp], l234_dt))
        bpsb = ctx.enter_context(nc.sbuf_tensor([128, 4], F32))
        h1sb = ctx.enter_context(nc.sbuf_tensor([128, 2, 2, NBMAX], l234_dt))
        h2sb = ctx.enter_context(nc.sbuf_tensor([128, 2, NBMAX], l234_dt))
        h3sb = ctx.enter_context(nc.sbuf_tensor([65, 2, NBMAX], l234_dt))
        osb = ctx.enter_context(nc.sbuf_tensor([10, BC], F32))
        warm = ctx.enter_context(nc.sbuf_tensor([1, 513], BF16))
        dump_a = ctx.enter_context(nc.sbuf_tensor([1, 16], BF16))

        w2v = wpsb[:, 0:256].rearrange("p (c o) -> p c o", c=2)
        w3v = wpsb[:, 256:320]
        w4v = wpsb[0:65, 320:330]
        b1v = bpsb[:, 0:2]
        b2v = bpsb[:, 2:3]
        b3v = bpsb[0:64, 3:4]

        ps1 = ctx.enter_context(nc.psum_tensor([128, 2, 2, NBMAX], F32))
        ps2 = ctx.enter_context(nc.psum_tensor([128, NBMAX], F32))
        ps3 = ctx.enter_context(nc.psum_tensor([64, NBMAX], F32))
        ps4 = ctx.enter_context(nc.psum_tensor([10, 2, NBMAX], F32))

        sx = [
            [ctx.enter_context(nc.semaphore(f"sx{t}_{i}")) for i in range(len(X_SPLITS[t]))]
            for t in range(NT)
        ]
        sw1 = [ctx.enter_context(nc.semaphore(f"sw1_{i}")) for i in range(3)]
        swr = ctx.enter_context(nc.semaphore("swr"))
        sm = ctx.enter_context(nc.semaphore("sm"))
        s2 = ctx.enter_context(nc.semaphore("s2"))
        sa = ctx.enter_context(nc.semaphore("sa"))
        sv = ctx.enter_context(nc.semaphore("sv"))
        sof = ctx.enter_context(nc.semaphore("sof"))
        sg = ctx.enter_context(nc.semaphore("sg"))

        block = ctx.enter_context(nc.Block())

        def make_xd(eng):
            def xd(t, i):
                c0, c1 = X_SPLITS[t][i]
                eng.dma_start(
                    out=xsb[:, col(t, c0) : col(t, c1)],
                    in_=xt_d[:, col(t, c0) : col(t, c1)],
                ).then_inc(sx[t][i], 16)
            return xd

        # w1 pieces: [0] = all of m0; [1] = m1 chunks 0-2; [2] = m1 chunks 3-6
        def w1d(eng, piece):
            lo, hi, m = [(0, NKC, 0), (0, 3, 1), (3, NKC, 1)][piece]
            eng.dma_start(
                out=w1sb[:, m, lo:hi, :], in_=w1_d[:, m, lo * 128 : hi * 128]
            ).then_inc(sw1[piece], 16)

        @block.sync
        def _(sy):
            xd = make_xd(sy)
            w1d(sy, 0)
            xd(0, 1)
            xd(0, 3)
            xd(0, 5)
            w1d(sy, 1)
            xd(1, 0)
            xd(1, 2)
            xd(2, 1)
            xd(2, 3)
            xd(3, 0)
            xd(4, 0)
            for t in range(NT):
                sy.wait_ge(sa, POS_A[("out", t)])
                sy.dma_start(
                    out=out_d[:, OFF[t] : OFF[t] + TS[t]],
                    in_=osb[:, OFF[t] : OFF[t] + TS[t]],
                ).then_inc(sof, 16)
            sy.wait_ge(sof, 16 * NT)

        @block.scalar
        def _(se):
            xd = make_xd(se)
            xd(0, 0)
            xd(0, 2)
            xd(0, 4)
            xd(0, 6)
            w1d(se, 2)
            se.dma_start(out=wpsb[:], in_=wp_d[:]).then_inc(swr, 16)
            se.dma_start(out=bpsb[:], in_=bp_d[:]).then_inc(swr, 16)
            xd(1, 1)
            xd(1, 3)
            xd(2, 0)
            xd(2, 2)
            xd(3, 1)
            xd(4, 1)
            se.wait_ge(sg, 1)
            se.activation(dump_a[:], warm[:, 0:16], RELU)  # preload relu table
            se.wait_ge(swr, 32)  # biases resident
            for kind, t in ACT_ORDER:
                st = t % 2
                ts = TS[t]
                if kind == "r0":
                    if t >= 2:
                        se.wait_ge(s2, POS_PE[("B", t - 2)])  # h1 set free
                    se.wait_ge(sm, 2 * t + 1)
                    se.activation(
                        h1sb[:, st, 0, 0:ts], ps1[:, st, 0, 0:ts], RELU,
                        bias=b1v[:, 0:1],
                    ).then_inc(sa, 1)
                elif kind == "h3":
                    se.wait_ge(s2, POS_PE[("C", t)])
                    se.activation(
                        h3sb[0:64, st, 0:ts], ps3[:, 0:ts], RELU, bias=b3v[:],
                    ).then_inc(sa, 1)
                else:  # out
                    se.wait_ge(s2, POS_PE[("D", t)])
                    se.copy(
                        osb[:, OFF[t] : OFF[t] + ts], ps4[:, t % 2, 0:ts]
                    ).then_inc(sa, 1)

        @block.vector
        def _(ve):
            ve.memset(warm[:], 0.125).then_inc(sg, 1)
            ve.memset(h3sb[64:65, :, :], 1.0).then_inc(sg, 1)
            ve.wait_ge(swr, 32)  # biases resident
            for kind, t in DVE_ORDER:
                st = t % 2
                ts = TS[t]
                if kind == "r1":
                    if t >= 2:
                        ve.wait_ge(s2, POS_PE[("B", t - 2)])  # h1 set free
                    ve.wait_ge(sm, 2 * t + 2)
                    ve.tensor_scalar(
                        h1sb[:, st, 1, 0:ts], ps1[:, st, 1, 0:ts], b1v[:, 1:2],
                        0.0, ADD, MAX,
                    ).then_inc(sv, 1)
                else:  # h2
                    ve.wait_ge(s2, POS_PE[("B", t)])
                    ve.tensor_scalar(
                        h2sb[:, st, 0:ts], ps2[:, 0:ts], b2v[:], 0.0, ADD, MAX
                    ).then_inc(sv, 1)

        @block.tensor
        def _(te):
            te.wait_ge(sg, 1)
            for _i in range(N_WARM_MM):
                te.matmul(ps2[0:1, :], warm[:, 0:1], warm[:, 1:513],
                          start=True, stop=True)

            def mm1(t, m, c):
                st = t % 2
                ts = TS[t]
                r = te.matmul(
                    ps1[:, st, m, 0:ts],
                    w1sb[:, m, c, :],
                    xsb[:, col(t, c) : col(t, c) + ts],
                    start=(c == 0),
                    stop=(c == NKC - 1),
                )
                if c == NKC - 1:
                    r.then_inc(sm, 1)

            def emit_L1(t):
                if t >= 2:
                    te.wait_ge(sa, POS_A[("r0", t - 2)])     # ps1[st,0] free
                    te.wait_ge(sv, POS_V[("r1", t - 2)])     # ps1[st,1] free
                if t == 0:
                    # m-outer: the m0 pass tracks chunk arrivals one matmul
                    # per chunk; the m1 pass reuses resident data.
                    te.wait_ge(sw1[0], 16)
                    for c in range(NKC):
                        te.wait_ge(sx[0][c], 16)
                        mm1(t, 0, c)
                    for c in range(NKC):
                        if c == 0:
                            te.wait_ge(sw1[1], 16)
                        elif c == 3:
                            te.wait_ge(sw1[2], 16)
                        mm1(t, 1, c)
                else:
                    # pair-interleaved: both m halves per arriving x piece.
                    for i, (c0, c1) in enumerate(X_SPLITS[t]):
                        te.wait_ge(sx[t][i], 16)
                        for m in range(2):
                            for c in range(c0, c1):
                                mm1(t, m, c)

            for kind, t in PE_ORDER:
                st = t % 2
                ts = TS[t]
                if kind == "A":
                    emit_L1(t)
                elif kind == "B":
                    if t == 0:
                        te.wait_ge(swr, 32)
                    te.wait_ge(sa, POS_A[("r0", t)])
                    if t >= 1:
                        te.wait_ge(sv, POS_V[("h2", t - 1)])  # ps2 free
                    te.matmul(
                        ps2[:, 0:ts], w2v[:, 0, :], h1sb[:, st, 0, 0:ts],
                        start=True, stop=False,
                    )
                    te.wait_ge(sv, POS_V[("r1", t)])
                    te.matmul(
                        ps2[:, 0:ts], w2v[:, 1, :], h1sb[:, st, 1, 0:ts],
                        start=False, stop=True,
                    ).then_inc(s2, 1)
                elif kind == "C":
                    te.wait_ge(sv, POS_V[("h2", t)])
                    if t >= 1:
                        te.wait_ge(sa, POS_A[("h3", t - 1)])  # ps3 free
                    te.matmul(
                        ps3[:, 0:ts], w3v[:], h2sb[:, st, 0:ts],
                        start=True, stop=True,
                    ).then_inc(s2, 1)
                else:
                    te.wait_ge(sa, POS_A[("h3", t)])
                    if t == 0:
                        te.wait_ge(sg, 2)  # h3 ones row written
                    if t >= 2:
                        te.wait_ge(sa, POS_A[("out", t - 2)])  # ps4 set free
                    te.matmul(
                        ps4[:, t % 2, 0:ts], w4v[:], h3sb[:, st, 0:ts],
                        start=True, stop=True,
                    ).then_inc(s2, 1)

    return nc


def _np_dt(dt):
    if dt == BF16:
        return ml_dtypes.bfloat16
    if dt == FP16:
        return np.float16
    return np.float32


def prepare_inputs(x, conv_w, w1, b1, w2, b2, w3, b3, w4, b4,
                   l1_dt=FP16, l234_dt=FP16):
    w1v = np.ascontiguousarray(w1.T).reshape(26, 26, 256)
    w1e = np.zeros((28, 28, 256), dtype=np.float32)
    for di in range(3):
        for dj in range(3):
            w1e[di : di + 26, dj : dj + 26, :] += conv_w[di, dj] * w1v
    w1e = w1e.reshape(784, 256)
    # [KC, 2, NKC*128]: contraction chunk on partitions, m-half major.
    w1t = np.ascontiguousarray(
        w1e.reshape(NKC, KC, 2, 128).transpose(1, 2, 0, 3)
    ).reshape(KC, 2, NKC * 128).astype(_np_dt(l1_dt))

    w2t = np.ascontiguousarray(w2.T).reshape(2, 128, 128).transpose(1, 0, 2)
    wpack = np.zeros((128, 256 + 64 + 10), dtype=np.float32)
    wpack[:, 0:256] = w2t.reshape(128, 256)
    wpack[:, 256:320] = w3.T
    wpack[0:64, 320:330] = w4.T
    wpack[64, 320:330] = b4  # ones row in h3 adds the L4 bias in-matmul
    wpack = wpack.astype(_np_dt(l234_dt))

    bpack = np.zeros((128, 4), dtype=np.float32)
    bpack[:, 0:2] = b1.reshape(2, 128).T
    bpack[:, 2] = b2
    bpack[0:64, 3] = b3

    shared = {"w1e": w1t, "wpack": wpack, "bpack": bpack}
    np_l1 = _np_dt(l1_dt)
    in_maps = []
    for m in range(N_CORES):
        xc = x[m * BC : (m + 1) * BC]
        blocks = []
        for t in range(NT):
            xb = xc[OFF[t] : OFF[t] + TS[t]]
            blocks.append(
                np.ascontiguousarray(
                    xb.reshape(TS[t], NKC, KC).transpose(2, 1, 0)
                ).reshape(KC, NKC * TS[t])
            )
        xt = np.concatenate(blocks, axis=1).astype(np_l1)
        in_maps.append({"xt": xt, **shared})
    return in_maps



_PROGRAM = None


def _get_program():
    global _PROGRAM
    if _PROGRAM is None:
        _PROGRAM = build_program()
    return _PROGRAM


def kernel(x, conv_w, w1, b1, w2, b2, w3, b3, w4, b4):
    from concourse import bass_utils

    args = [x, conv_w, w1, b1, w2, b2, w3, b3, w4, b4]
    x, conv_w, w1, b1, w2, b2, w3, b3, w4, b4 = [
        np.asarray(a, dtype=np.float32) for a in args
    ]
    nc = _get_program()
    in_maps = prepare_inputs(x, conv_w, w1, b1, w2, b2, w3, b3, w4, b4)
    res = bass_utils.run_bass_kernel_spmd(nc, in_maps, list(range(N_CORES)))
    out = np.concatenate(
        [np.ascontiguousarray(res.results[m]["outT"].T) for m in range(N_CORES)],
        axis=0,
    )
    return out.astype(np.float32)


# revision 57
# speedup vs baseline: 1.0816x; 1.0816x over previous
"""Trainium2 Bass kernel for nn_DigitConvolutionalModel (dense CNN -> MLP).

Pure data parallel over 8 NeuronCores (2048 samples each). The 3x3 conv is
linear, so the host folds it into the first FC layer (W1e = C @ w1.T), making
the whole network a 4-layer MLP computed in transposed orientation (features
on partitions, batch on the free dim) in fp16 (psum fp32, ~5e-4 rel err):

    outT = w4e.T @ relu3(w3t.T @ relu(w2t.T @ relu(W1e.T @ xT + b1) + b2) + b3)

where relu3 appends a constant ones row so w4e's extra row adds b4 inside the
L4 matmul; the final output is a pure fp32 copy PSUM->SBUF then DMA.

Raw bass with manual semaphores. Both HW DGE queues (sync + scalar) carry the
input stream in need order so arrivals pace the PE; warmup matmuls keep the
PE busy from the preamble's end (the DVFS governor only reaches 2.4GHz after
a long continuous-busy stretch, and the PE must never idle >0.5us). Batch
tiles are [512,512,512,256,256]: the two half-width trailing tiles halve
every op in the serial end chain (relu/copy on ACT/DVE + L2-L4 matmuls),
shrinking the critical tail after the last x byte lands. All PSUM accesses
stay bank-aligned at offset 0 (this hardware faults on intra-bank offsets).

PE op order (A=L1, B=L2, C=L3, D=L4):
  A0 A1 B0 A2 C0 B1 A3 D0 C1 B2 A4 D1 C2 B3 D2 C3 B4 D3 C4 D4
L1 relu halves run on ACT (m=0) and DVE (m=1) concurrently; h2 on DVE;
h3 and the out copies on ACT. s2 counts PE tail ops (B/C/D) in PE order.
"""

from contextlib import ExitStack

import ml_dtypes
import numpy as np

import concourse.bass as bass
import concourse.mybir as mybir

N_CORES = 8
B = 16384
BC = B // N_CORES
KC = 112
NKC = 7

TS = [512, 512, 512, 256, 256]          # batch tile sizes
OFF = [sum(TS[:t]) for t in range(len(TS))]
NT = len(TS)
NBMAX = max(TS)

F32 = mybir.dt.float32
BF16 = mybir.dt.bfloat16
FP16 = mybir.dt.float16
RELU = mybir.ActivationFunctionType.Relu
ADD = mybir.AluOpType.add
MAX = mybir.AluOpType.max

N_WARM_MM = 13

# x DMA chunk splits per tile (in units of the tile's contraction chunks).
X_SPLITS = [
    [(c, c + 1) for c in range(NKC)],
    [(0, 2), (2, 4), (4, 6), (6, 7)],
    [(0, 2), (2, 4), (4, 6), (6, 7)],
    [(0, 4), (4, 7)],
    [(0, 4), (4, 7)],
]

PE_ORDER = [
    ("A", 0), ("A", 1), ("B", 0), ("A", 2), ("C", 0), ("B", 1), ("A", 3),
    ("D", 0), ("C", 1), ("B", 2), ("A", 4), ("D", 1), ("C", 2), ("B", 3),
    ("D", 2), ("C", 3), ("B", 4), ("D", 3), ("C", 4), ("D", 4),
]
TAILS = [(k, t) for (k, t) in PE_ORDER if k != "A"]
POS_PE = {op: i + 1 for i, op in enumerate(TAILS)}  # s2 thresholds

# ACT ops (r0 = L1 relu first half; h3; out = ps4->osb copy), in readiness
# order (by the PE-emission index of each op's producer).
ACT_ORDER = [
    ("r0", 0), ("r0", 1), ("r0", 2), ("h3", 0), ("r0", 3), ("out", 0),
    ("h3", 1), ("r0", 4), ("out", 1), ("h3", 2), ("out", 2), ("h3", 3),
    ("out", 3), ("h3", 4), ("out", 4),
]
POS_A = {op: i + 1 for i, op in enumerate(ACT_ORDER)}  # sa thresholds

# DVE ops (r1 = L1 relu second half; h2), in readiness order.
DVE_ORDER = [
    ("r1", 0), ("r1", 1), ("h2", 0), ("r1", 2), ("h2", 1), ("r1", 3),
    ("h2", 2), ("r1", 4), ("h2", 3), ("h2", 4),
]
POS_V = {op: i + 1 for i, op in enumerate(DVE_ORDER)}  # sv thresholds


def build_program(l1_dt=FP16, l234_dt=FP16):
    nc = bass.Bass()

    n_wp = 256 + 64 + 10
    ncol = NKC * BC

    xt_d = nc.declare_dram_parameter("xt", [KC, ncol], l1_dt, isOutput=False)
    w1_d = nc.declare_dram_parameter("w1e", [KC, 2, NKC * 128], l1_dt, isOutput=False)
    wp_d = nc.declare_dram_parameter("wpack", [128, n_wp], l234_dt, isOutput=False)
    bp_d = nc.declare_dram_parameter("bpack", [128, 4], F32, isOutput=False)
    out_d = nc.declare_dram_parameter("outT", [10, BC], F32, isOutput=True)

    def col(t, c):
        return NKC * OFF[t] + c * TS[t]

    ctx = ExitStack()
    with ctx:
        xsb = ctx.enter_context(nc.sbuf_tensor([KC, ncol], l1_dt))
        w1sb = ctx.enter_context(nc.sbuf_tensor([KC, 2, NKC, 128], l1_dt))
        wpsb = ctx.enter_context(nc.sbuf_tensor([128, n_wp], l234_dt))
        bpsb = ctx.enter_context(nc.sbuf_tensor([128, 4], F32))
        h1sb = ctx.enter_context(nc.sbuf_tensor([128, 2, 2, NBMAX], l234_dt))
        h2sb = ctx.enter_context(nc.sbuf_tensor([128, 2, NBMAX], l234_dt))
        h3sb = ctx.enter_context(nc.sbuf_tensor([65, 2, NBMAX], l234_dt))
        osb = ctx.enter_context(nc.sbuf_tensor([10, BC], F32))
        warm = ctx.enter_context(nc.sbuf_tensor([1, 513], BF16))
        dump_a = ctx.enter_context(nc.sbuf_tensor([1, 16], BF16))

        w2v = wpsb[:, 0:256].rearrange("p (c o) -> p c o", c=2)
        w3v = wpsb[:, 256:320]
        w4v = wpsb[0:65, 320:330]
        b1v = bpsb[:, 0:2]
        b2v = bpsb[:, 2:3]
        b3v = bpsb[0:64, 3:4]

        ps1 = ctx.enter_context(nc.psum_tensor([128, 2, 2, NBMAX], F32))
        ps2 = ctx.enter_context(nc.psum_tensor([128, NBMAX], F32))
        ps3 = ctx.enter_context(nc.psum_tensor([64, NBMAX], F32))
        ps4 = ctx.enter_context(nc.psum_tensor([10, 2, NBMAX], F32))

        sx = [
            [ctx.enter_context(nc.semaphore(f"sx{t}_{i}")) for i in range(len(X_SPLITS[t]))]
            for t in range(NT)
        ]
        sw1 = [ctx.enter_context(nc.semaphore(f"sw1_{i}")) for i in range(3)]
        swr = ctx.enter_context(nc.semaphore("swr"))
        sm = ctx.enter_context(nc.semaphore("sm"))
        s2 = ctx.enter_context(nc.semaphore("s2"))
        sa = ctx.enter_context(nc.semaphore("sa"))
        sv = ctx.enter_context(nc.semaphore("sv"))
        sof = ctx.enter_context(nc.semaphore("sof"))
        sg = ctx.enter_context(nc.semaphore("sg"))

        block = ctx.enter_context(nc.Block())

        def make_xd(eng):
            def xd(t, i):
                c0, c1 = X_SPLITS[t][i]
                eng.dma_start(
                    out=xsb[:, col(t, c0) : col(t, c1)],
                    in_=xt_d[:, col(t, c0) : col(t, c1)],
                ).then_inc(sx[t][i], 16)
            return xd

        # w1 pieces: [0] = all of m0; [1] = m1 chunks 0-2; [2] = m1 chunks 3-6
        def w1d(eng, piece):
            lo, hi, m = [(0, NKC, 0), (0, 3, 1), (3, NKC, 1)][piece]
            eng.dma_start(
                out=w1sb[:, m, lo:hi, :], in_=w1_d[:, m, lo * 128 : hi * 128]
            ).then_inc(sw1[piece], 16)

        @block.sync
        def _(sy):
            xd = make_xd(sy)
            w1d(sy, 0)
            xd(0, 1)
            xd(0, 3)
            xd(0, 5)
            w1d(sy, 1)
            xd(1, 0)
            xd(1, 2)
            xd(2, 1)
            xd(2, 3)
            xd(3, 0)
            xd(4, 0)
            for t in range(NT):
                sy.wait_ge(sa, POS_A[("out", t)])
                sy.dma_start(
                    out=out_d[:, OFF[t] : OFF[t] + TS[t]],
                    in_=osb[:, OFF[t] : OFF[t] + TS[t]],
                ).then_inc(sof, 16)
            sy.wait_ge(sof, 16 * NT)

        @block.scalar
        def _(se):
            xd = make_xd(se)
            xd(0, 0)
            xd(0, 2)
            xd(0, 4)
            xd(0, 6)
            w1d(se, 2)
            se.dma_start(out=wpsb[:], in_=wp_d[:]).then_inc(swr, 16)
            se.dma_start(out=bpsb[:], in_=bp_d[:]).then_inc(swr, 16)
            xd(1, 1)
            xd(1, 3)
            xd(2, 0)
            xd(2, 2)
            xd(3, 1)
            xd(4, 1)
            se.wait_ge(sg, 1)
            se.activation(dump_a[:], warm[:, 0:16], RELU)  # preload relu table
            se.wait_ge(swr, 32)  # biases resident
            for kind, t in ACT_ORDER:
                st = t % 2
                ts = TS[t]
                if kind == "r0":
                    if t >= 2:
                        se.wait_ge(s2, POS_PE[("B", t - 2)])  # h1 set free
                    se.wait_ge(sm, 2 * t + 1)
                    se.activation(
                        h1sb[:, st, 0, 0:ts], ps1[:, st, 0, 0:ts], RELU,
                        bias=b1v[:, 0:1],
                    ).then_inc(sa, 1)
                elif kind == "h3":
                    se.wait_ge(s2, POS_PE[("C", t)])
                    se.activation(
                        h3sb[0:64, st, 0:ts], ps3[:, 0:ts], RELU, bias=b3v[:],
                    ).then_inc(sa, 1)
                else:  # out
                    se.wait_ge(s2, POS_PE[("D", t)])
                    se.copy(
                        osb[:, OFF[t] : OFF[t] + ts], ps4[:, t % 2, 0:ts]
                    ).then_inc(sa, 1)

        @block.vector
        def _(ve):
            ve.memset(warm[:], 0.125).then_inc(sg, 1)
            ve.memset(h3sb[64:65, :, :], 1.0).then_inc(sg, 1)
            ve.wait_ge(swr, 32)  # biases resident
            for kind, t in DVE_ORDER:
                st = t % 2
                ts = TS[t]
                if kind == "r1":
                    if t >= 2:
                        ve.wait_ge(s2, POS_PE[("B", t - 2)])  # h1 set free
                    ve.wait_ge(sm, 2 * t + 2)
                    ve.tensor_scalar(
                        h1sb[:, st, 1, 0:ts], ps1[:, st, 1, 0:ts], b1v[:, 1:2],
                        0.0, ADD, MAX,
                    ).then_inc(sv, 1)
                else:  # h2
                    ve.wait_ge(s2, POS_PE[("B", t)])
                    ve.tensor_scalar(
                        h2sb[:, st, 0:ts], ps2[:, 0:ts], b2v[:], 0.0, ADD, MAX
                    ).then_inc(sv, 1)

        @block.tensor
        def _(te):
            te.wait_ge(sg, 1)
            for _i in range(N_WARM_MM):
                te.matmul(ps2[0:1, :], warm[:, 0:1], warm[:, 1:513],
                          start=True, stop=True)

            def mm1(t, m, c):
                st = t % 2
                ts = TS[t]
                r = te.matmul(
                    ps1[:, st, m, 0:ts],
                    w1sb[:, m, c, :],
                    xsb[:, col(t, c) : col(t, c) + ts],
                    start=(c == 0),
                    stop=(c == NKC - 1),
                )
                if c == NKC - 1:
                    r.then_inc(sm, 1)

            def emit_L1(t):
                if t >= 2:
                    te.wait_ge(sa, POS_A[("r0", t - 2)])     # ps1[st,0] free
                    te.wait_ge(sv, POS_V[("r1", t - 2)])     # ps1[st,1] free
                if t == 0:
                    # m-outer: the m0 pass tracks chunk arrivals one matmul
                    # per chunk; the m1 pass reuses resident data.
                    te.wait_ge(sw1[0], 16)
                    for c in range(NKC):
                        te.wait_ge(sx[0][c], 16)
                        mm1(t, 0, c)
                    for c in range(NKC):
                        if c == 0:
                            te.wait_ge(sw1[1], 16)
                        elif c == 3:
                            te.wait_ge(sw1[2], 16)
                        mm1(t, 1, c)
                else:
                    # pair-interleaved: both m halves per arriving x piece.
                    for i, (c0, c1) in enumerate(X_SPLITS[t]):
                        te.wait_ge(sx[t][i], 16)
                        for m in range(2):
                            for c in range(c0, c1):
                                mm1(t, m, c)

            for kind, t in PE_ORDER:
                st = t % 2
                ts = TS[t]
                if kind == "A":
                    emit_L1(t)
                elif kind == "B":
                    if t == 0:
                        te.wait_ge(swr, 32)
                    te.wait_ge(sa, POS_A[("r0", t)])
                    if t >= 1:
                        te.wait_ge(sv, POS_V[("h2", t - 1)])  # ps2 free
                    te.matmul(
                        ps2[:, 0:ts], w2v[:, 0, :], h1sb[:, st, 0, 0:ts],
                        start=True, stop=False,
                    )
                    te.wait_ge(sv, POS_V[("r1", t)])
                    te.matmul(
                        ps2[:, 0:ts], w2v[:, 1, :], h1sb[:, st, 1, 0:ts],
                        start=False, stop=True,
                    ).then_inc(s2, 1)
                elif kind == "C":
                    te.wait_ge(sv, POS_V[("h2", t)])
                    if t >= 1:
                        te.wait_ge(sa, POS_A[("h3", t - 1)])  # ps3 free
                    te.matmul(
                        ps3[:, 0:ts], w3v[:], h2sb[:, st, 0:ts],
                        start=True, stop=True,
                    ).then_inc(s2, 1)
                else:
                    te.wait_ge(sa, POS_A[("h3", t)])
                    if t == 0:
                        te.wait_ge(sg, 2)  # h3 ones row written
                    if t >= 2:
                        te.wait_ge(sa, POS_A[("out", t - 2)])  # ps4 set free
                    te.matmul(
                        ps4[:, t % 2, 0:ts], w4v[:], h3sb[:, st, 0:ts],
                        start=True, stop=True,
                    ).then_inc(s2, 1)

    return nc


def _np_dt(dt):
    if dt == BF16:
        return ml_dtypes.bfloat16
    if dt == FP16:
        return np.float16
    return np.float32


def prepare_inputs(x, conv_w, w1, b1, w2, b2, w3, b3, w4, b4,
                   l1_dt=FP16, l234_dt=FP16):
    w1v = np.ascontiguousarray(w1.T).reshape(26, 26, 256)
    w1e = np.zeros((28, 28, 256), dtype=np.float32)
    for di in range(3):
        for dj in range(3):
            w1e[di : di + 26, dj : dj + 26, :] += conv_w[di, dj] * w1v
    w1e = w1e.reshape(784, 256)
    # [KC, 2, NKC*128]: contraction chunk on partitions, m-half major.
    w1t = np.ascontiguousarray(
        w1e.reshape(NKC, KC, 2, 128).transpose(1, 2, 0, 3)
    ).reshape(KC, 2, NKC * 128).astype(_np_dt(l1_dt))

    w2t = np.ascontiguousarray(w2.T).reshape(2, 128, 128).transpose(1, 0, 2)
    wpack = np.zeros((128, 256 + 64 + 10), dtype=np.float32)
    wpack[:, 0:256] = w2t.reshape(128, 256)
    wpack[:, 256:320] = w3.T
    wpack[0:64, 320:330] = w4.T
    wpack[64, 320:330] = b4  # ones row in h3 adds the L4 bias in-matmul
    wpack = wpack.astype(_np_dt(l234_dt))

    bpack = np.zeros((128, 4), dtype=np.float32)
    bpack[:, 0:2] = b1.reshape(2, 128).T
    bpack[:, 2] = b2
    bpack[0:64, 3] = b3

    shared = {"w1e": w1t, "wpack": wpack, "bpack": bpack}
    np_l1 = _np_dt(l1_dt)
    in_maps = []
    for m in range(N_CORES):
        xc = x[m * BC : (m + 1) * BC]
        blocks = []
        for t in range(NT):
            xb = xc[OFF[t] : OFF[t] + TS[t]]
            blocks.append(
                np.ascontiguousarray(
                    xb.reshape(TS[t], NKC, KC).transpose(2, 1, 0)
                ).reshape(KC, NKC * TS[t])
            )
        xt = np.concatenate(blocks, axis=1).astype(np_l1)
        in_maps.append({"xt": xt, **shared})
    return in_maps



_PROGRAM = None


def _get_program():
    global _PROGRAM
    if _PROGRAM is None:
        _PROGRAM = build_program()
    return _PROGRAM


def kernel(x, conv_w, w1, b1, w2, b2, w3, b3, w4, b4):
    from concourse import bass_utils

    args = [x, conv_w, w1, b1, w2, b2, w3, b3, w4, b4]
    x, conv_w, w1, b1, w2, b2, w3, b3, w4, b4 = [
        np.asarray(a, dtype=np.float32) for a in args
    ]
    nc = _get_program()
    in_maps = prepare_inputs(x, conv_w, w1, b1, w2, b2, w3, b3, w4, b4)
    res = bass_utils.run_bass_kernel_spmd(nc, in_maps, list(range(N_CORES)))
    out = np.concatenate(
        [np.ascontiguousarray(res.results[m]["outT"].T) for m in range(N_CORES)],
        axis=0,
    )
    return out.astype(np.float32)
